# revision 36
# baseline (speedup 1.0000x reference)
"""Trainium2 Bass kernel for nn_Attention_13554916786722.

Reference computation (per (b, h) head; B=4, H=8, S=2048, D=512):
    score[d, e]  = sum_s q[s, d] * k[s, e] / sqrt(H)      # (D, D) logits
    atten_map    = score                                   # returned pre-softmax
    attn         = softmax(score, axis=-1)
    out[d, s]    = sum_e attn[d, e] * v[s, e]              # (D, S)

Sharding: batch*heads (32 independent heads) across 8 cores, 4 heads each.

Device-side design (per head):
  - q is pre-scaled by 1/sqrt(H) on the host, so PSUM holds final logits.
  - m1 computes score[d, e] = (Q^T K) with d on PSUM partitions; softmax
    statistics are free-dim ops: row max via DVE reduce (negated, used as
    the exp bias), row sum via the Exp activation's accum_out.
  - The logits here reach +-400 (q/k columns are correlated), so a real
    row max is required; exp(logit - rowmax) <= 1 also lets P live in fp16.
  - m2 contracts over e, needing P^T as lhsT: 16 PE transposes per head
    (128x128, via identity) + PSUM->SBUF copies.
  - q, k fed in fp16 (10-bit mantissa preserves logit ranking; bf16 would
    flip near-tied argmaxes), v in fp16, all outputs fp16, upcast on host.
"""

import os
import sys

for _p in ("/opt/trn_rl_repo",):
    if _p not in sys.path and os.path.isdir(_p):
        sys.path.append(_p)

import numpy as np

import concourse.bacc as bacc
import concourse.mybir as mybir
import concourse.tile as tile
from concourse.bass_utils import run_bass_kernel_spmd
from concourse.masks import make_identity

B, H, S, D = 4, 8, 2048, 512
N_CORES = 8
HPC = B * H // N_CORES  # heads per core
P = 128
S_T = S // P   # 16 s-tiles
E_T = D // P   # 4 e-tiles
D_T = D // P   # 4 d-tiles
SBLK = 512     # matmul free-dim block over s
SCALE = 1.0 / np.sqrt(H)

F16 = mybir.dt.float16
F32 = mybir.dt.float32

_CACHE = {}

# tuning knobs (A/B tested on hardware)
T_MODE = os.environ.get("KRN_T_MODE", "pe")  # 'pe' | 'dma' transpose path
WARM_MMS = int(os.environ.get("KRN_WARM", "24"))  # tiny PE warmup matmuls
TS_SPLIT = os.environ.get("KRN_TS_SPLIT", "0") == "1"  # out-scale DVE/ACT split


def _build():
    nc = bacc.Bacc("TRN2", target_bir_lowering=False, debug=False,
                   num_devices=N_CORES)
    qd = nc.declare_dram_parameter("q", [HPC, S, D], F16, isOutput=False)
    kd = nc.declare_dram_parameter("k", [HPC, S, D], F16, isOutput=False)
    vtd = nc.declare_dram_parameter("vT", [HPC, D, S], F16, isOutput=False)
    scored = nc.declare_dram_parameter("score", [HPC, D, D], F16, isOutput=True)
    outd = nc.declare_dram_parameter("out", [HPC, D, S], F16, isOutput=True)

    with tile.TileContext(nc) as tc:
        with (
            tc.tile_pool(name="qp", bufs=2) as qp,
            tc.tile_pool(name="kp", bufs=2) as kp,
            tc.tile_pool(name="vp", bufs=2) as vp,
            tc.tile_pool(name="pp", bufs=2) as pp,
            tc.tile_pool(name="ptp", bufs=2) as ptp,
            tc.tile_pool(name="rp", bufs=2) as rp,
            tc.tile_pool(name="smallp", bufs=4) as smallp,
            tc.tile_pool(name="sp", bufs=3) as sp,
            tc.tile_pool(name="op", bufs=4) as op,
            tc.tile_pool(name="cp", bufs=1) as cp,
            tc.tile_pool(name="ps1", bufs=3, space="PSUM") as ps1p,
            tc.tile_pool(name="ps2", bufs=3, space="PSUM") as ps2p,
            tc.tile_pool(name="pst", bufs=2, space="PSUM") as pstp,
        ):
            ident = cp.tile([P, P], F16)
            make_identity(nc, ident[:])
            deferred = []

            # keep the PE's activity monitor busy during the initial DMA
            # ramp so real matmuls run at the unthrottled clock. Tiny
            # free-dim matmuls are issue-bound (~cheap) but count as busy.
            if WARM_MMS:
                warmw = cp.tile([P, 64], F16)
                nc.vector.memset(warmw[:], 0.5)
                wps = ps2p.tile([P, 64], F32, tag="ps2")
                for _ in range(WARM_MMS):
                    nc.tensor.matmul(wps[:32], warmw[:, :32], warmw[:],
                                     start=True, stop=True)

            for h in range(HPC):
                q_sb = qp.tile([P, S_T, D], F16, tag="q")
                k_sb = kp.tile([P, S_T, D], F16, tag="k")
                vt_sb = vp.tile([P, E_T, S], F16, tag="v")
                p_sb = pp.tile([P, D_T, D], F16, tag="p")
                pt_sb = ptp.tile([P, E_T, D], F16, tag="pt")
                rsinv = rp.tile([P, D_T], F32, tag="r")

                qr = qd[h].rearrange("(t p) d -> p t d", p=P)
                kr = kd[h].rearrange("(t p) d -> p t d", p=P)
                vr = vtd[h].rearrange("(t p) s -> p t s", p=P)
                for c in range(4):
                    nc.sync.dma_start(q_sb[:, 4 * c:4 * c + 4, :],
                                      qr[:, 4 * c:4 * c + 4, :])
                    nc.sync.dma_start(k_sb[:, 4 * c:4 * c + 4, :],
                                      kr[:, 4 * c:4 * c + 4, :])

                scr = scored[h].rearrange("(t p) e -> p t e", p=P)

                def m1(dt):
                    ps1 = ps1p.tile([P, D], F32, tag="ps1")
                    for st in range(S_T):
                        nc.tensor.matmul(
                            ps1[:],
                            q_sb[:, st, P * dt:P * dt + P],
                            k_sb[:, st, :],
                            start=(st == 0),
                            stop=(st == S_T - 1),
                        )
                    m1_post(dt, ps1)

                def m1_post(dt, ps1):
                    nbias = smallp.tile([P, 1], F32, tag="nbias")
                    nc.vector.tensor_reduce(
                        nbias[:], ps1[:], mybir.AxisListType.X,
                        mybir.AluOpType.max, negate=True)
                    sc = sp.tile([P, D], F16, tag="s")
                    nc.vector.tensor_copy(sc[:], ps1[:])
                    nc.sync.dma_start(scr[:, dt, :], sc[:])
                    rowsum = smallp.tile([P, 1], F32, tag="rowsum")
                    nc.scalar.activation(
                        p_sb[:, dt, :], ps1[:],
                        mybir.ActivationFunctionType.Exp,
                        bias=nbias[:], scale=1.0, accum_out=rowsum[:],
                    )
                    nc.vector.reciprocal(rsinv[:, dt:dt + 1], rowsum[:])

                def transpose_p(dt):
                    for et in range(E_T):
                        if T_MODE == "dma":
                            nc.scalar.dma_start_transpose(
                                pt_sb[:, et, P * dt:P * dt + P],
                                p_sb[:, dt, P * et:P * et + P])
                        else:
                            tps = pstp.tile([P, P], F16, tag="tps")
                            nc.tensor.transpose(
                                tps[:], p_sb[:, dt, P * et:P * et + P],
                                ident[:])
                            nc.vector.tensor_copy(
                                pt_sb[:, et, P * dt:P * dt + P], tps[:])

                our = outd[h].rearrange("(t p) s -> p t s", p=P)

                def m2(dt, defer_tail=False):
                    ou = op.tile([P, S], F16, tag="o")

                    def scale_sb(sb, ps2, dt=dt, ou=ou, rs=rsinv):
                        if TS_SPLIT and sb % 2 == 1:
                            nc.scalar.mul(
                                ou[:, SBLK * sb:SBLK * sb + SBLK],
                                ps2[:], rs[:, dt:dt + 1])
                        else:
                            nc.vector.tensor_scalar_mul(
                                ou[:, SBLK * sb:SBLK * sb + SBLK],
                                ps2[:], rs[:, dt:dt + 1])

                    tail_ps = []
                    for sb in range(S // SBLK):
                        ps2 = ps2p.tile([P, SBLK], F32, tag="ps2",
                                        name=f"ps2_{sb}")
                        for et in range(E_T):
                            nc.tensor.matmul(
                                ps2[:],
                                pt_sb[:, et, P * dt:P * dt + P],
                                vt_sb[:, et, SBLK * sb:SBLK * sb + SBLK],
                                start=(et == 0),
                                stop=(et == E_T - 1),
                            )
                        if defer_tail and sb >= 2:
                            tail_ps.append((sb, ps2))
                        else:
                            scale_sb(sb, ps2)

                    if defer_tail:
                        def finalize(tail_ps=tail_ps, dt=dt, ou=ou,
                                     our_=our):
                            for sb, ps2 in tail_ps:
                                scale_sb(sb, ps2)
                            nc.sync.dma_start(our_[:, dt, :], ou[:])
                        deferred.append(finalize)
                    else:
                        nc.sync.dma_start(our[:, dt, :], ou[:])

                def vload():
                    for c in range(4):
                        nc.sync.dma_start(vt_sb[:, c, :], vr[:, c, :])

                # software-pipelined emission keeps the PE free of stalls:
                # softmax stats for tile dt overlap matmuls of other tiles.
                m1(0)
                vload()
                # flush the previous head's deferred output finalizes now:
                # the softmax-stat DVE ops for this head were emitted first,
                # so PSUM for m1 recycles without waiting on output scaling.
                for fin in deferred:
                    fin()
                deferred.clear()
                m1(1)
                transpose_p(0)
                m1(2)
                transpose_p(1)
                m1(3)
                m2(0)
                transpose_p(2)
                m2(1)
                transpose_p(3)
                m2(2)
                m2(3, defer_tail=(h < HPC - 1))
            for fin in deferred:
                fin()
            deferred.clear()

    nc.compile()
    return nc


def kernel(q, k, v):
    if "nc" not in _CACHE:
        _CACHE["nc"] = _build()
    nc = _CACHE["nc"]

    qf = np.asarray(q, dtype=np.float32).reshape(B * H, S, D) * SCALE
    kf = np.asarray(k, dtype=np.float32).reshape(B * H, S, D)
    vf = np.asarray(v, dtype=np.float32).reshape(B * H, S, D)

    in_maps = []
    for c in range(N_CORES):
        sl = slice(HPC * c, HPC * (c + 1))
        in_maps.append({
            "q": qf[sl].astype(np.float16),
            "k": kf[sl].astype(np.float16),
            "vT": np.ascontiguousarray(
                vf[sl].transpose(0, 2, 1)).astype(np.float16),
        })

    res = run_bass_kernel_spmd(
        nc, in_maps, core_ids=list(range(N_CORES)),
        trace_cores=list(range(N_CORES)),
    )
    _CACHE["last_results"] = res

    out = np.stack([np.asarray(res.results[c]["out"]) for c in range(N_CORES)])
    out = out.astype(np.float32).reshape(B, H, D, S)
    att = np.stack([np.asarray(res.results[c]["score"]) for c in range(N_CORES)])
    att = att.astype(np.float32).reshape(B, H, D, D)
    return out, att


# revision 37
# speedup vs baseline: 1.0522x; 1.0522x over previous
"""Trainium2 Bass kernel for nn_Attention_13554916786722.

Reference computation (per (b, h) head; B=4, H=8, S=2048, D=512):
    score[d, e]  = sum_s q[s, d] * k[s, e] / sqrt(H)      # (D, D) logits
    atten_map    = score                                   # returned pre-softmax
    attn         = softmax(score, axis=-1)
    out[d, s]    = sum_e attn[d, e] * v[s, e]              # (D, S)

Sharding: batch*heads (32 independent heads) across 8 cores, 4 heads each.

Device-side design (per head):
  - q is pre-scaled by 1/sqrt(H) on the host, so PSUM holds final logits.
  - m1 computes score[d, e] = (Q^T K) with d on PSUM partitions; softmax
    statistics are free-dim ops: row max via DVE reduce (negated, used as
    the exp bias), row sum via the Exp activation's accum_out.
  - The logits here reach +-400 (q/k columns are correlated), so a real
    row max is required; exp(logit - rowmax) <= 1 also lets P live in fp16.
  - m2 contracts over e, needing P^T as lhsT: 16 PE transposes per head
    (128x128, via identity) + PSUM->SBUF copies.
  - q, k fed in fp16 (10-bit mantissa preserves logit ranking; bf16 would
    flip near-tied argmaxes), v in fp16, all outputs fp16, upcast on host.
"""

import os
import sys

for _p in ("/opt/trn_rl_repo",):
    if _p not in sys.path and os.path.isdir(_p):
        sys.path.append(_p)

import numpy as np

import concourse.bacc as bacc
import concourse.mybir as mybir
import concourse.tile as tile
from concourse.bass_utils import run_bass_kernel_spmd
from concourse.masks import make_identity

B, H, S, D = 4, 8, 2048, 512
N_CORES = 8
HPC = B * H // N_CORES  # heads per core
P = 128
S_T = S // P   # 16 s-tiles
E_T = D // P   # 4 e-tiles
D_T = D // P   # 4 d-tiles
SBLK = 512     # matmul free-dim block over s
SCALE = 1.0 / np.sqrt(H)

F16 = mybir.dt.float16
F32 = mybir.dt.float32

_CACHE = {}

# tuning knobs (A/B tested on hardware)
T_MODE = os.environ.get("KRN_T_MODE", "pe")  # 'pe' | 'dma' transpose path
WARM_MMS = int(os.environ.get("KRN_WARM", "24"))  # tiny PE warmup matmuls
TS_SPLIT = os.environ.get("KRN_TS_SPLIT", "0") == "1"  # out-scale DVE/ACT split


def _build():
    nc = bacc.Bacc("TRN2", target_bir_lowering=False, debug=False,
                   num_devices=N_CORES)
    qd = nc.declare_dram_parameter("q", [HPC, S, D], F16, isOutput=False)
    kd = nc.declare_dram_parameter("k", [HPC, S, D], F16, isOutput=False)
    vtd = nc.declare_dram_parameter("vT", [HPC, D, S], F16, isOutput=False)
    scored = nc.declare_dram_parameter("score", [HPC, D, D], F16, isOutput=True)
    outd = nc.declare_dram_parameter("out", [HPC, D, S], F16, isOutput=True)

    with tile.TileContext(nc) as tc:
        with (
            tc.tile_pool(name="qp", bufs=2) as qp,
            tc.tile_pool(name="kp", bufs=2) as kp,
            tc.tile_pool(name="vp", bufs=2) as vp,
            tc.tile_pool(name="pp", bufs=2) as pp,
            tc.tile_pool(name="ptp", bufs=2) as ptp,
            tc.tile_pool(name="rp", bufs=2) as rp,
            tc.tile_pool(name="smallp", bufs=4) as smallp,
            tc.tile_pool(name="sp", bufs=3) as sp,
            tc.tile_pool(name="op", bufs=4) as op,
            tc.tile_pool(name="cp", bufs=1) as cp,
            tc.tile_pool(name="ps1", bufs=3, space="PSUM") as ps1p,
            tc.tile_pool(name="ps2", bufs=3, space="PSUM") as ps2p,
            tc.tile_pool(name="pst", bufs=2, space="PSUM") as pstp,
        ):
            ident = cp.tile([P, P], F16)
            make_identity(nc, ident[:])
            deferred = []

            # keep the PE's activity monitor busy during the initial DMA
            # ramp so real matmuls run at the unthrottled clock. Tiny
            # free-dim matmuls are issue-bound (~cheap) but count as busy.
            if WARM_MMS:
                warmw = cp.tile([P, 64], F16)
                nc.vector.memset(warmw[:], 0.5)
                wps = ps2p.tile([P, 64], F32, tag="ps2")
                for _ in range(WARM_MMS):
                    nc.tensor.matmul(wps[:32], warmw[:, :32], warmw[:],
                                     start=True, stop=True)

            for h in range(HPC):
                q_sb = qp.tile([P, S_T, D], F16, tag="q")
                k_sb = kp.tile([P, S_T, D], F16, tag="k")
                vt_sb = vp.tile([P, E_T, S], F16, tag="v")
                p_sb = pp.tile([P, D_T, D], F16, tag="p")
                pt_sb = ptp.tile([P, E_T, D], F16, tag="pt")
                rsinv = rp.tile([P, D_T], F32, tag="r")

                qr = qd[h].rearrange("(t p) d -> p t d", p=P)
                kr = kd[h].rearrange("(t p) d -> p t d", p=P)
                vr = vtd[h].rearrange("(t p) s -> p t s", p=P)
                for c in range(4):
                    nc.sync.dma_start(q_sb[:, 4 * c:4 * c + 4, :],
                                      qr[:, 4 * c:4 * c + 4, :])
                    nc.sync.dma_start(k_sb[:, 4 * c:4 * c + 4, :],
                                      kr[:, 4 * c:4 * c + 4, :])

                scr = scored[h].rearrange("(t p) e -> p t e", p=P)

                def m1(dt):
                    ps1 = ps1p.tile([P, D], F32, tag="ps1")
                    for st in range(S_T):
                        nc.tensor.matmul(
                            ps1[:],
                            q_sb[:, st, P * dt:P * dt + P],
                            k_sb[:, st, :],
                            start=(st == 0),
                            stop=(st == S_T - 1),
                        )
                    m1_post(dt, ps1)

                def m1_post(dt, ps1):
                    nbias = smallp.tile([P, 1], F32, tag="nbias")
                    nc.vector.tensor_reduce(
                        nbias[:], ps1[:], mybir.AxisListType.X,
                        mybir.AluOpType.max, negate=True)
                    sc = sp.tile([P, D], F16, tag="s")
                    nc.vector.tensor_copy(sc[:], ps1[:])
                    nc.sync.dma_start(scr[:, dt, :], sc[:])
                    rowsum = smallp.tile([P, 1], F32, tag="rowsum")
                    nc.scalar.activation(
                        p_sb[:, dt, :], ps1[:],
                        mybir.ActivationFunctionType.Exp,
                        bias=nbias[:], scale=1.0, accum_out=rowsum[:],
                    )
                    nc.vector.reciprocal(rsinv[:, dt:dt + 1], rowsum[:])

                def transpose_p(dt):
                    for et in range(E_T):
                        if T_MODE == "dma":
                            nc.scalar.dma_start_transpose(
                                pt_sb[:, et, P * dt:P * dt + P],
                                p_sb[:, dt, P * et:P * et + P])
                        else:
                            tps = pstp.tile([P, P], F16, tag="tps")
                            nc.tensor.transpose(
                                tps[:], p_sb[:, dt, P * et:P * et + P],
                                ident[:])
                            nc.vector.tensor_copy(
                                pt_sb[:, et, P * dt:P * dt + P], tps[:])

                our = outd[h].rearrange("(t p) s -> p t s", p=P)

                def m2(dt, defer_tail=False):
                    ou = op.tile([P, S], F16, tag="o")

                    def scale_sb(sb, ps2, dt=dt, ou=ou, rs=rsinv):
                        if TS_SPLIT and sb % 2 == 1:
                            nc.scalar.mul(
                                ou[:, SBLK * sb:SBLK * sb + SBLK],
                                ps2[:], rs[:, dt:dt + 1])
                        else:
                            nc.vector.tensor_scalar_mul(
                                ou[:, SBLK * sb:SBLK * sb + SBLK],
                                ps2[:], rs[:, dt:dt + 1])

                    tail_ps = []
                    for sb in range(S // SBLK):
                        ps2 = ps2p.tile([P, SBLK], F32, tag="ps2",
                                        name=f"ps2_{sb}")
                        for et in range(E_T):
                            nc.tensor.matmul(
                                ps2[:],
                                pt_sb[:, et, P * dt:P * dt + P],
                                vt_sb[:, et, SBLK * sb:SBLK * sb + SBLK],
                                start=(et == 0),
                                stop=(et == E_T - 1),
                            )
                        if defer_tail and sb >= 2:
                            tail_ps.append((sb, ps2))
                        else:
                            scale_sb(sb, ps2)

                    if defer_tail:
                        def finalize(tail_ps=tail_ps, dt=dt, ou=ou,
                                     our_=our):
                            for sb, ps2 in tail_ps:
                                scale_sb(sb, ps2)
                            nc.sync.dma_start(our_[:, dt, :], ou[:])
                        deferred.append(finalize)
                    elif h == HPC - 1 and dt == D_T - 1:
                        # kernel tail: ship the last tile in small pieces so
                        # the exit barrier isn't waiting on one 512KB DMA
                        for sb in range(S // SBLK):
                            nc.sync.dma_start(
                                our[:, dt, SBLK * sb:SBLK * sb + SBLK],
                                ou[:, SBLK * sb:SBLK * sb + SBLK])
                    else:
                        nc.sync.dma_start(our[:, dt, :], ou[:])

                def vload():
                    for c in range(4):
                        nc.sync.dma_start(vt_sb[:, c, :], vr[:, c, :])

                # software-pipelined emission keeps the PE free of stalls:
                # softmax stats for tile dt overlap matmuls of other tiles.
                m1(0)
                vload()
                # flush the previous head's deferred output finalizes now:
                # the softmax-stat DVE ops for this head were emitted first,
                # so PSUM for m1 recycles without waiting on output scaling.
                for fin in deferred:
                    fin()
                deferred.clear()
                m1(1)
                transpose_p(0)
                m1(2)
                transpose_p(1)
                m1(3)
                m2(0)
                transpose_p(2)
                m2(1)
                transpose_p(3)
                m2(2)
                m2(3, defer_tail=(h < HPC - 1))
            for fin in deferred:
                fin()
            deferred.clear()

    nc.compile()
    return nc


def kernel(q, k, v):
    if "nc" not in _CACHE:
        _CACHE["nc"] = _build()
    nc = _CACHE["nc"]

    qf = np.asarray(q, dtype=np.float32).reshape(B * H, S, D) * SCALE
    kf = np.asarray(k, dtype=np.float32).reshape(B * H, S, D)
    vf = np.asarray(v, dtype=np.float32).reshape(B * H, S, D)

    in_maps = []
    for c in range(N_CORES):
        sl = slice(HPC * c, HPC * (c + 1))
        in_maps.append({
            "q": qf[sl].astype(np.float16),
            "k": kf[sl].astype(np.float16),
            "vT": np.ascontiguousarray(
                vf[sl].transpose(0, 2, 1)).astype(np.float16),
        })

    res = run_bass_kernel_spmd(
        nc, in_maps, core_ids=list(range(N_CORES)),
        trace_cores=list(range(N_CORES)),
    )
    _CACHE["last_results"] = res

    out = np.stack([np.asarray(res.results[c]["out"]) for c in range(N_CORES)])
    out = out.astype(np.float32).reshape(B, H, D, S)
    att = np.stack([np.asarray(res.results[c]["score"]) for c in range(N_CORES)])
    att = att.astype(np.float32).reshape(B, H, D, D)
    return out, att


# revision 38
# speedup vs baseline: 1.0589x; 1.0064x over previous
"""Trainium2 Bass kernel for nn_Attention_13554916786722.

Reference computation (per (b, h) head; B=4, H=8, S=2048, D=512):
    score[d, e]  = sum_s q[s, d] * k[s, e] / sqrt(H)      # (D, D) logits
    atten_map    = score                                   # returned pre-softmax
    attn         = softmax(score, axis=-1)
    out[d, s]    = sum_e attn[d, e] * v[s, e]              # (D, S)

Sharding: batch*heads (32 independent heads) across 8 cores, 4 heads each.

Device-side design (per head):
  - q is pre-scaled by 1/sqrt(H) on the host, so PSUM holds final logits.
  - m1 computes score[d, e] = (Q^T K) with d on PSUM partitions; softmax
    statistics are free-dim ops: row max via DVE reduce (negated, used as
    the exp bias), row sum via the Exp activation's accum_out.
  - The logits here reach +-400 (q/k columns are correlated), so a real
    row max is required; exp(logit - rowmax) <= 1 also lets P live in fp16.
  - m2 contracts over e, needing P^T as lhsT: 16 PE transposes per head
    (128x128, via identity) + PSUM->SBUF copies.
  - q, k fed in fp16 (10-bit mantissa preserves logit ranking; bf16 would
    flip near-tied argmaxes), v in fp16, all outputs fp16, upcast on host.
"""

import os
import sys

for _p in ("/opt/trn_rl_repo",):
    if _p not in sys.path and os.path.isdir(_p):
        sys.path.append(_p)

import numpy as np

import concourse.bacc as bacc
import concourse.mybir as mybir
import concourse.tile as tile
from concourse.bass_utils import run_bass_kernel_spmd
from concourse.masks import make_identity

B, H, S, D = 4, 8, 2048, 512
N_CORES = 8
HPC = B * H // N_CORES  # heads per core
P = 128
S_T = S // P   # 16 s-tiles
E_T = D // P   # 4 e-tiles
D_T = D // P   # 4 d-tiles
SBLK = 512     # matmul free-dim block over s
SCALE = 1.0 / np.sqrt(H)

F16 = mybir.dt.float16
F32 = mybir.dt.float32

_CACHE = {}

# tuning knobs (A/B tested on hardware)
T_MODE = os.environ.get("KRN_T_MODE", "pe")  # 'pe' | 'dma' transpose path
WARM_MMS = int(os.environ.get("KRN_WARM", "24"))  # tiny PE warmup matmuls
TS_SPLIT = os.environ.get("KRN_TS_SPLIT", "0") == "1"  # out-scale DVE/ACT split


def _build():
    nc = bacc.Bacc("TRN2", target_bir_lowering=False, debug=False,
                   num_devices=N_CORES)
    qd = nc.declare_dram_parameter("q", [HPC, S, D], F16, isOutput=False)
    kd = nc.declare_dram_parameter("k", [HPC, S, D], F16, isOutput=False)
    vtd = nc.declare_dram_parameter("vT", [HPC, D, S], F16, isOutput=False)
    scored = nc.declare_dram_parameter("score", [HPC, D, D], F16, isOutput=True)
    outd = nc.declare_dram_parameter("out", [HPC, D, S], F16, isOutput=True)

    with tile.TileContext(nc) as tc:
        with (
            tc.tile_pool(name="qp", bufs=2) as qp,
            tc.tile_pool(name="kp", bufs=2) as kp,
            tc.tile_pool(name="vp", bufs=2) as vp,
            tc.tile_pool(name="pp", bufs=2) as pp,
            tc.tile_pool(name="ptp", bufs=2) as ptp,
            tc.tile_pool(name="rp", bufs=2) as rp,
            tc.tile_pool(name="smallp", bufs=4) as smallp,
            tc.tile_pool(name="sp", bufs=3) as sp,
            tc.tile_pool(name="op", bufs=4) as op,
            tc.tile_pool(name="cp", bufs=1) as cp,
            tc.tile_pool(name="ps1", bufs=3, space="PSUM") as ps1p,
            tc.tile_pool(name="ps2", bufs=3, space="PSUM") as ps2p,
            tc.tile_pool(name="pst", bufs=2, space="PSUM") as pstp,
        ):
            ident = cp.tile([P, P], F16)
            make_identity(nc, ident[:])
            deferred = []

            # keep the PE's activity monitor busy during the initial DMA
            # ramp so real matmuls run at the unthrottled clock. Tiny
            # free-dim matmuls are issue-bound (~cheap) but count as busy.
            if WARM_MMS:
                warmw = cp.tile([P, 64], F16)
                nc.vector.memset(warmw[:], 0.5)
                wps = ps2p.tile([P, 64], F32, tag="ps2")
                for _ in range(WARM_MMS):
                    nc.tensor.matmul(wps[:32], warmw[:, :32], warmw[:],
                                     start=True, stop=True)

            for h in range(HPC):
                q_sb = qp.tile([P, S_T, D], F16, tag="q")
                k_sb = kp.tile([P, S_T, D], F16, tag="k")
                vt_sb = vp.tile([P, E_T, S], F16, tag="v")
                p_sb = pp.tile([P, D_T, D], F16, tag="p")
                pt_sb = ptp.tile([P, E_T, D], F16, tag="pt")
                rsinv = rp.tile([P, D_T], F32, tag="r")

                qr = qd[h].rearrange("(t p) d -> p t d", p=P)
                kr = kd[h].rearrange("(t p) d -> p t d", p=P)
                vr = vtd[h].rearrange("(t p) s -> p t s", p=P)
                nch = 4 if h else 8  # finer chunks for the first head's ramp
                cw = S_T // nch
                for c in range(nch):
                    nc.sync.dma_start(q_sb[:, cw * c:cw * c + cw, :],
                                      qr[:, cw * c:cw * c + cw, :])
                    nc.sync.dma_start(k_sb[:, cw * c:cw * c + cw, :],
                                      kr[:, cw * c:cw * c + cw, :])

                scr = scored[h].rearrange("(t p) e -> p t e", p=P)

                def m1(dt):
                    ps1 = ps1p.tile([P, D], F32, tag="ps1")
                    for st in range(S_T):
                        nc.tensor.matmul(
                            ps1[:],
                            q_sb[:, st, P * dt:P * dt + P],
                            k_sb[:, st, :],
                            start=(st == 0),
                            stop=(st == S_T - 1),
                        )
                    m1_post(dt, ps1)

                def m1_post(dt, ps1):
                    nbias = smallp.tile([P, 1], F32, tag="nbias")
                    nc.vector.tensor_reduce(
                        nbias[:], ps1[:], mybir.AxisListType.X,
                        mybir.AluOpType.max, negate=True)
                    sc = sp.tile([P, D], F16, tag="s")
                    nc.vector.tensor_copy(sc[:], ps1[:])
                    nc.sync.dma_start(scr[:, dt, :], sc[:])
                    rowsum = smallp.tile([P, 1], F32, tag="rowsum")
                    nc.scalar.activation(
                        p_sb[:, dt, :], ps1[:],
                        mybir.ActivationFunctionType.Exp,
                        bias=nbias[:], scale=1.0, accum_out=rowsum[:],
                    )
                    nc.vector.reciprocal(rsinv[:, dt:dt + 1], rowsum[:])

                def transpose_p(dt):
                    for et in range(E_T):
                        if T_MODE == "dma":
                            nc.scalar.dma_start_transpose(
                                pt_sb[:, et, P * dt:P * dt + P],
                                p_sb[:, dt, P * et:P * et + P])
                        else:
                            tps = pstp.tile([P, P], F16, tag="tps")
                            nc.tensor.transpose(
                                tps[:], p_sb[:, dt, P * et:P * et + P],
                                ident[:])
                            nc.vector.tensor_copy(
                                pt_sb[:, et, P * dt:P * dt + P], tps[:])

                our = outd[h].rearrange("(t p) s -> p t s", p=P)

                def m2(dt, defer_tail=False):
                    ou = op.tile([P, S], F16, tag="o")

                    def scale_sb(sb, ps2, dt=dt, ou=ou, rs=rsinv):
                        if TS_SPLIT and sb % 2 == 1:
                            nc.scalar.mul(
                                ou[:, SBLK * sb:SBLK * sb + SBLK],
                                ps2[:], rs[:, dt:dt + 1])
                        else:
                            nc.vector.tensor_scalar_mul(
                                ou[:, SBLK * sb:SBLK * sb + SBLK],
                                ps2[:], rs[:, dt:dt + 1])

                    tail_ps = []
                    for sb in range(S // SBLK):
                        ps2 = ps2p.tile([P, SBLK], F32, tag="ps2",
                                        name=f"ps2_{sb}")
                        for et in range(E_T):
                            nc.tensor.matmul(
                                ps2[:],
                                pt_sb[:, et, P * dt:P * dt + P],
                                vt_sb[:, et, SBLK * sb:SBLK * sb + SBLK],
                                start=(et == 0),
                                stop=(et == E_T - 1),
                            )
                        if defer_tail and sb >= 2:
                            tail_ps.append((sb, ps2))
                        else:
                            scale_sb(sb, ps2)

                    if defer_tail:
                        def finalize(tail_ps=tail_ps, dt=dt, ou=ou,
                                     our_=our):
                            for sb, ps2 in tail_ps:
                                scale_sb(sb, ps2)
                            nc.sync.dma_start(our_[:, dt, :], ou[:])
                        deferred.append(finalize)
                    elif h == HPC - 1 and dt == D_T - 1:
                        # kernel tail: ship the last tile in small pieces so
                        # the exit barrier isn't waiting on one 512KB DMA
                        for sb in range(S // SBLK):
                            nc.sync.dma_start(
                                our[:, dt, SBLK * sb:SBLK * sb + SBLK],
                                ou[:, SBLK * sb:SBLK * sb + SBLK])
                    else:
                        nc.sync.dma_start(our[:, dt, :], ou[:])

                def vload():
                    for c in range(4):
                        nc.sync.dma_start(vt_sb[:, c, :], vr[:, c, :])

                # software-pipelined emission keeps the PE free of stalls:
                # softmax stats for tile dt overlap matmuls of other tiles.
                m1(0)
                vload()
                # flush the previous head's deferred output finalizes now:
                # the softmax-stat DVE ops for this head were emitted first,
                # so PSUM for m1 recycles without waiting on output scaling.
                for fin in deferred:
                    fin()
                deferred.clear()
                m1(1)
                transpose_p(0)
                m1(2)
                transpose_p(1)
                m1(3)
                m2(0)
                transpose_p(2)
                m2(1)
                transpose_p(3)
                m2(2)
                m2(3, defer_tail=(h < HPC - 1))
            for fin in deferred:
                fin()
            deferred.clear()

    nc.compile()
    return nc


def kernel(q, k, v):
    if "nc" not in _CACHE:
        _CACHE["nc"] = _build()
    nc = _CACHE["nc"]

    qf = np.asarray(q, dtype=np.float32).reshape(B * H, S, D) * SCALE
    kf = np.asarray(k, dtype=np.float32).reshape(B * H, S, D)
    vf = np.asarray(v, dtype=np.float32).reshape(B * H, S, D)

    in_maps = []
    for c in range(N_CORES):
        sl = slice(HPC * c, HPC * (c + 1))
        in_maps.append({
            "q": qf[sl].astype(np.float16),
            "k": kf[sl].astype(np.float16),
            "vT": np.ascontiguousarray(
                vf[sl].transpose(0, 2, 1)).astype(np.float16),
        })

    res = run_bass_kernel_spmd(
        nc, in_maps, core_ids=list(range(N_CORES)),
        trace_cores=list(range(N_CORES)),
    )
    _CACHE["last_results"] = res

    out = np.stack([np.asarray(res.results[c]["out"]) for c in range(N_CORES)])
    out = out.astype(np.float32).reshape(B, H, D, S)
    att = np.stack([np.asarray(res.results[c]["score"]) for c in range(N_CORES)])
    att = att.astype(np.float32).reshape(B, H, D, D)
    return out, att
